# revision 1
# baseline (speedup 1.0000x reference)
"""Trainium2 Bass kernel for nn_CayleyNet (gnn_message_passing), 8 NeuronCores.

Strategy (graph/data parallel, per sharding hint):
- Nodes sharded 2500/core (padded to 2560 = 20 tiles x 128 partitions).
- Edges partitioned by scatter-destination; per destination-tile groups of
  GT x 128 edge slots (host-sorted/padded). Two orderings: O1 (scatter=row,
  gather=col; used by the B apply) and O2 (scatter=col, gather=row; Jacobi).
- CayleyNet edge weights depend only on one endpoint (tmp_left[row]), so every
  sparse op is an *unweighted* adjacency apply + per-node complex scalings:
      B y = -h*tl (.) (A1 @ y) + b_dia (.) y
      Jacobi: yk' = A2 @ (h*tl (.) yk) + b_j
- Per propagation: AllGather bf16 node-state table (re||im, 512B rows) ->
  dma_gather source rows -> one-hot S matmuls on TensorE (segment-sum into
  PSUM, f32) -> fused DVE combines (scalar_tensor_tensor with per-partition
  node scalars).
- Dense W / Wc matmuls in bf16 via PE with PE transposes between node-major
  and feature-major layouts. Small weights replicated.
- Device computes x2 (feature-major, f32). Host does tanh-score / top-k /
  weighted mean / final linear (~0.25% of FLOPs; top-k selection).
"""
import numpy as np
import ml_dtypes

import concourse.bass as bass
import concourse.bacc as bacc
import concourse.mybir as mybir
import concourse.tile as tile
from concourse.bass_utils import run_bass_kernel_spmd

# ---- problem constants (hardcoded per spec) ----
N = 20000
E = 320000
FEAT = 128
HID = 128
OUT = 10
R = 2
K = 3
RATIO = 0.9
NCORES = 8
NLOC = 2500
NT = 20                  # node tiles per core
NLOC_PAD = NT * 128      # 2560
ZROWS = NCORES * NLOC_PAD
F = 128                  # feature width
F2 = 2 * F               # re||im row width of the z table
ET = 128                 # edges per tile

BF16 = mybir.dt.bfloat16
F32 = mybir.dt.float32
I16 = mybir.dt.int16

_cache = {}


# ----------------------------------------------------------------------------
# host preprocessing
# ----------------------------------------------------------------------------

def _zrow(g):
    """z-table row for global node id g (tile-major local layout)."""
    c = g // NLOC
    return c * NLOC_PAD + (g - c * NLOC)


def _build_edge_tables(row, col):
    """Per ordering/core: gather-idx (wrapped int16) + one-hot S blocks.

    Returns (GT, tabs) where tabs[name] = (idx_wrapped [NCORES,128,cols] i16,
    S [NCORES, NT, 128, GT*128] bf16).
    """
    # determine GT uniformly: max edges landing in any (core, dst-tile) group
    maxg = 0
    for dst in (row, col):
        loc = dst % NLOC
        gid = (dst // NLOC) * NT + loc // 128   # global group id (core*NT + tile)
        cnt = np.bincount(gid, minlength=NCORES * NT)
        maxg = max(maxg, int(cnt.max()))
    GT = int(np.ceil(maxg / ET))
    ESLOTS = NT * GT * ET

    tabs = {}
    for name, dst, src in (("O1", row, col), ("O2", col, row)):
        order = np.argsort(dst, kind="stable")
        dst_s, src_s = dst[order], src[order]
        idx_all = np.zeros((NCORES, ESLOTS), np.int32)
        S_all = np.zeros((NCORES, NT * GT, ET, ET), np.float32)
        zr = _zrow(src_s)
        d_loc = dst_s % NLOC
        d_core = dst_s // NLOC
        d_tile = d_loc // 128
        d_slot = d_loc - d_tile * 128
        for c in range(NCORES):
            m = d_core == c
            dt_c, ds_c, zr_c = d_tile[m], d_slot[m], zr[m]
            for g in range(NT):
                gm = dt_c == g
                nd = int(gm.sum())
                assert nd <= GT * ET
                base = g * GT * ET
                idx_all[c, base:base + nd] = zr_c[gm]
                ts = np.arange(nd)
                S_all[c, g * GT + ts // ET, ts % ET, ds_c[gm]] = 1.0
        # wrap idx: logical i -> [i%16, i//16], replicated to 128 partitions
        w = idx_all.reshape(NCORES, ESLOTS // 16, 16).transpose(0, 2, 1)
        w = np.tile(w, (1, 8, 1)).astype(np.int16)
        # S layout for streaming: [NT, 128e, GT*128d]
        S_flat = (S_all.reshape(NCORES, NT, GT, ET, ET)
                  .transpose(0, 1, 3, 2, 4)
                  .reshape(NCORES, NT, ET, GT * ET)
                  .astype(ml_dtypes.bfloat16))
        tabs[name] = (w, S_flat)
    return GT, tabs


def _shard_cols(v):
    """[N] -> [NCORES, 128, NT] per-node columns (node (t,p) -> [:, p, t])."""
    out = np.zeros((NCORES, 128, NT), np.float32)
    pad = np.zeros(NCORES * NLOC_PAD, np.float32)
    for c in range(NCORES):
        pad[c * NLOC_PAD: c * NLOC_PAD + NLOC] = v[c * NLOC: (c + 1) * NLOC]
    lp = pad.reshape(NCORES, NT, 128)
    return lp.transpose(0, 2, 1).copy()


def _node_major(x):
    """[N, F] f32 -> [NCORES, 128, NT, F]: node (t,p) at [c, p, t, :]."""
    out = np.zeros((NCORES, NT, 128, x.shape[1]), np.float32)
    for c in range(NCORES):
        out[c].reshape(NLOC_PAD, -1)[:NLOC] = x[c * NLOC:(c + 1) * NLOC]
    return out.transpose(0, 2, 1, 3).copy()


def _scales(deg, h, alpha):
    """Per-conv per-node scale columns. Returns dict of [N] f32 arrays.

    s = B post-scale, d = b_dia, g = Jacobi pre-scale; gs = g*s and gd = g*d
    fold the first Jacobi gather operand z = g.(s.u + d.y) into one chain.
    """
    l = (deg - alpha).astype(np.float64)
    tl = 1.0 / (h * l + 1j)
    s = -h * tl
    d = tl * (h * l - 1j)
    g = h * tl
    gs = g * s
    gd = g * d
    out = {}
    for nm, v in (("s", s), ("d", d), ("g", g), ("gs", gs), ("gd", gd)):
        out[nm + "_re"] = np.real(v)
        out[nm + "_im"] = np.imag(v)
        out["n" + nm + "_im"] = -np.imag(v)
    return out


SCAL_NAMES = ["s_re", "s_im", "ns_im", "d_re", "d_im", "nd_im",
              "g_re", "g_im", "ng_im",
              "gs_re", "gs_im", "ngs_im", "gd_re", "gd_im", "ngd_im"]
NSCAL = len(SCAL_NAMES)


# ----------------------------------------------------------------------------
# kernel builder
# ----------------------------------------------------------------------------

def _build(GT):
    ESLOTS = NT * GT * ET
    ICOLS = ESLOTS // 16
    GCOLS = GT * ET // 16     # idx cols per group

    nc = bacc.Bacc("TRN2", target_bir_lowering=False, debug=False,
                   num_devices=NCORES)

    xz_in = nc.dram_tensor("xz", [128, NT, F2], BF16, kind="ExternalInput")
    y0_in = nc.dram_tensor("y0", [128, NT * F], F32, kind="ExternalInput")
    idx1_in = nc.dram_tensor("idx1", [128, ICOLS], I16, kind="ExternalInput")
    idx2_in = nc.dram_tensor("idx2", [128, ICOLS], I16, kind="ExternalInput")
    s1_in = nc.dram_tensor("s1", [NT, 128, GT * ET], BF16, kind="ExternalInput")
    s2_in = nc.dram_tensor("s2", [NT, 128, GT * ET], BF16, kind="ExternalInput")
    scal_in = nc.dram_tensor("scal", [128, 2 * NSCAL * NT], F32, kind="ExternalInput")
    wts_in = nc.dram_tensor("wts", [128, 10 * 128], BF16, kind="ExternalInput")
    ident_in = nc.dram_tensor("ident", [128, 128], BF16, kind="ExternalInput")
    xfeat_out = nc.dram_tensor("xfeat", [128, NT * F], F32, kind="ExternalOutput")

    s_dram = {"O1": s1_in, "O2": s2_in}

    with tile.TileContext(nc) as tc:
        with (
            tc.tile_pool(name="persist", bufs=1) as pp,
            tc.tile_pool(name="gpool", bufs=4) as gpool,
            tc.tile_pool(name="spool", bufs=8) as spool,
            tc.tile_pool(name="tmp", bufs=4) as tmpp,
            tc.tile_pool(name="prop_ps", bufs=5, space="PSUM") as prop_ps,
            tc.tile_pool(name="tr_ps", bufs=2, space="PSUM") as tr_ps,
            tc.tile_pool(name="mm_ps", bufs=1, space="PSUM") as mm_ps,
            tc.tile_pool(name="dram", bufs=1, space="DRAM") as dram,
        ):
            # ---- persistent SBUF state ----
            z_own = pp.tile([128, NT, F2], BF16)        # bf16 node state (re||im)
            y_re = pp.tile([128, NT, F], F32)
            y_im = pp.tile([128, NT, F], F32)
            b_re = pp.tile([128, NT, F], F32)
            b_im = pp.tile([128, NT, F], F32)
            out_acc = pp.tile([128, NT * F], F32)       # feature-major conv accum
            xT = pp.tile([128, NT * F], BF16)           # transposed input [c, n]
            yT_re = pp.tile([128, NT * F], BF16)
            yT_im = pp.tile([128, NT * F], BF16)
            idx_sb = {"O1": pp.tile([128, ICOLS], I16, name="idx_o1"),
                      "O2": pp.tile([128, ICOLS], I16, name="idx_o2")}
            scal_sb = pp.tile([128, 2 * NSCAL * NT], F32)
            wts_sb = pp.tile([128, 10 * 128], BF16)
            ident = pp.tile([128, 128], BF16)

            zin = dram.tile([NLOC_PAD, F2], BF16)
            # one Shared AllGather output per propagation (single-writer rule)
            ztabs = [dram.tile([ZROWS, F2], BF16, addr_space="Shared",
                               name=f"ztab{i}")
                     for i in range(2 * R * (1 + K))]
            prop_counter = [0]

            # ---- load constants ----
            nc.sync.dma_start(idx_sb["O1"][:], idx1_in[:])
            nc.sync.dma_start(idx_sb["O2"][:], idx2_in[:])
            nc.sync.dma_start(scal_sb[:], scal_in[:])
            nc.sync.dma_start(wts_sb[:], wts_in[:])
            nc.sync.dma_start(ident[:], ident_in[:])
            nc.sync.dma_start(z_own[:], xz_in[:])
            nc.sync.dma_start(y_re[:], y0_in[:])
            nc.vector.memset(y_im[:], 0.0)
            for t in range(NT):
                nc.sync.dma_start(zin[t * 128:(t + 1) * 128, :], z_own[:, t, :])

            def col(ci, name, t):
                k = ci * NSCAL + SCAL_NAMES.index(name)
                return scal_sb[:, k * NT + t: k * NT + t + 1]

            def wt(k):
                return wts_sb[:, k * 128:(k + 1) * 128]

            def prop(ordering, consumer):
                """One adjacency apply: AllGather z -> gather -> S matmuls.

                zin rows are written per-tile by the PREVIOUS prop's consumers
                (via zin_tile below), so only the AllGather remains here.
                """
                ztab = ztabs[prop_counter[0]]
                prop_counter[0] += 1
                nc.gpsimd.collective_compute(
                    "AllGather", mybir.AluOpType.bypass,
                    replica_groups=[list(range(NCORES))],
                    ins=[zin.opt()], outs=[ztab.opt()],
                )
                for g in range(NT):
                    gb = gpool.tile([128, GT, F2], BF16, tag="gbuf")
                    nc.gpsimd.dma_gather(
                        gb[:], ztab[:],
                        idx_sb[ordering][:, g * GCOLS:(g + 1) * GCOLS],
                        num_idxs=GT * ET, num_idxs_reg=GT * ET, elem_size=F2,
                        single_packet=False,
                    )
                    ssb = spool.tile([128, GT * ET], BF16, tag="schunk")
                    nc.sync.dma_start(ssb[:], s_dram[ordering][g])
                    ps = prop_ps.tile([128, F2], F32, tag="prop_ps")
                    for t in range(GT):
                        nc.tensor.matmul(ps[:], ssb[:, t * ET:(t + 1) * ET],
                                         gb[:, t, :],
                                         start=(t == 0), stop=(t == GT - 1))
                    consumer(g, ps)

            STT = nc.vector.scalar_tensor_tensor
            MUL = mybir.AluOpType.mult
            ADD = mybir.AluOpType.add
            COPY = mybir.ActivationFunctionType.Copy

            def smul(out_ap, in_ap, c_ap):
                """out = in * per-partition scalar, on the (idle) Scalar engine."""
                nc.scalar.activation(out_ap, in_ap, COPY, scale=c_ap)

            def zin_tile(g):
                """Push this dst tile's fresh z rows to the DRAM table input."""
                nc.sync.dma_start(zin[g * 128:(g + 1) * 128, :], z_own[:, g, :])

            def b_consumer(ci):
                def consume(g, ps):
                    u_re, u_im = ps[:, 0:F], ps[:, F:F2]
                    # critical path first: z = gs.u + gd.y (both complex prods)
                    tmp = tmpp.tile([128, F], F32, tag="ctmp")
                    smul(tmp[:], u_re, col(ci, "gs_re", g))
                    STT(tmp[:], u_im, col(ci, "ngs_im", g), tmp[:], MUL, ADD)
                    STT(tmp[:], y_re[:, g, :], col(ci, "gd_re", g), tmp[:], MUL, ADD)
                    STT(z_own[:, g, 0:F], y_im[:, g, :], col(ci, "ngd_im", g), tmp[:], MUL, ADD)
                    tmp2 = tmpp.tile([128, F], F32, tag="ctmp2")
                    smul(tmp2[:], u_im, col(ci, "gs_re", g))
                    STT(tmp2[:], u_re, col(ci, "gs_im", g), tmp2[:], MUL, ADD)
                    STT(tmp2[:], y_im[:, g, :], col(ci, "gd_re", g), tmp2[:], MUL, ADD)
                    STT(z_own[:, g, F:F2], y_re[:, g, :], col(ci, "gd_im", g), tmp2[:], MUL, ADD)
                    zin_tile(g)
                    # deferred: b = s.u + d.y (consumed by next prop's combines)
                    tmp3 = tmpp.tile([128, F], F32, tag="ctmp3")
                    smul(tmp3[:], u_re, col(ci, "s_re", g))
                    STT(tmp3[:], u_im, col(ci, "ns_im", g), tmp3[:], MUL, ADD)
                    STT(tmp3[:], y_re[:, g, :], col(ci, "d_re", g), tmp3[:], MUL, ADD)
                    STT(b_re[:, g, :], y_im[:, g, :], col(ci, "nd_im", g), tmp3[:], MUL, ADD)
                    tmp4 = tmpp.tile([128, F], F32, tag="ctmp4")
                    smul(tmp4[:], u_im, col(ci, "s_re", g))
                    STT(tmp4[:], u_re, col(ci, "s_im", g), tmp4[:], MUL, ADD)
                    STT(tmp4[:], y_im[:, g, :], col(ci, "d_re", g), tmp4[:], MUL, ADD)
                    STT(b_im[:, g, :], y_re[:, g, :], col(ci, "d_im", g), tmp4[:], MUL, ADD)
                return consume

            def jacobi_consumer(ci, last):
                def consume(g, ps):
                    u_re, u_im = ps[:, 0:F], ps[:, F:F2]
                    # y = u + b   (this is yk)
                    nc.vector.tensor_tensor(y_re[:, g, :], u_re, b_re[:, g, :], ADD)
                    nc.vector.tensor_tensor(y_im[:, g, :], u_im, b_im[:, g, :], ADD)
                    if last:
                        # z = bf16(y) for next B apply / Wc transposes
                        smul(z_own[:, g, 0:F], y_re[:, g, :], 1.0)
                        smul(z_own[:, g, F:F2], y_im[:, g, :], 1.0)
                    else:
                        # z = g (.) y   (next Jacobi gather operand)
                        tmp = tmpp.tile([128, F], F32, tag="ctmp")
                        smul(tmp[:], y_re[:, g, :], col(ci, "g_re", g))
                        STT(z_own[:, g, 0:F], y_im[:, g, :], col(ci, "ng_im", g), tmp[:], MUL, ADD)
                        tmp2 = tmpp.tile([128, F], F32, tag="ctmp2")
                        smul(tmp2[:], y_im[:, g, :], col(ci, "g_re", g))
                        STT(z_own[:, g, F:F2], y_re[:, g, :], col(ci, "g_im", g), tmp2[:], MUL, ADD)
                    zin_tile(g)
                return consume

            def transpose_to(dst, src_ap, t):
                """dst[:, t*128:(t+1)*128] = src_ap.T (both bf16)."""
                pt = tr_ps.tile([128, 128], BF16, tag="trps")
                nc.tensor.transpose(pt[:], src_ap, ident[:])
                nc.vector.tensor_copy(dst[:, t * 128:(t + 1) * 128], pt[:])

            def dense_chunks(lhs_ks, rhs_list, first):
                """out_acc[:, ch] (+)= sum_i lhsT(k_i) @ rhs_i[:, ch] (x2 if not first)."""
                nch = NT * F // 512
                for ch in range(nch):
                    sl = slice(ch * 512, (ch + 1) * 512)
                    ps = mm_ps.tile([128, 512], F32, tag="mmps")
                    for i, (k, rhs) in enumerate(zip(lhs_ks, rhs_list)):
                        nc.tensor.matmul(ps[:], wt(k), rhs[:, sl],
                                         start=(i == 0), stop=(i == len(lhs_ks) - 1))
                    if first:
                        nc.vector.tensor_copy(out_acc[:, sl], ps[:])
                    else:
                        STT(out_acc[:, sl], ps[:], 2.0, out_acc[:, sl], MUL, ADD)

            # ================= conv block =================
            for ci in range(2):
                wbase = ci * 5
                # xT = transpose(x_bf) from z re-halves
                for t in range(NT):
                    transpose_to(xT, z_own[:, t, 0:F], t)
                dense_chunks([wbase + 0], [xT], first=True)

                for j in range(R):
                    prop("O1", b_consumer(ci))
                    for it in range(K):
                        prop("O2", jacobi_consumer(ci, last=(it == K - 1)))
                    # yT from z halves (bf16 copies of y)
                    for t in range(NT):
                        transpose_to(yT_re, z_own[:, t, 0:F], t)
                        transpose_to(yT_im, z_own[:, t, F:F2], t)
                    dense_chunks([wbase + 1 + 2 * j, wbase + 2 + 2 * j],
                                 [yT_re, yT_im], first=False)

                if ci == 0:
                    # relu -> bf16, transpose back to node-major, reseed state
                    r_bf = pp.tile([128, NT * F], BF16, tag="rbf")
                    nc.vector.tensor_scalar_max(r_bf[:], out_acc[:], 0.0)
                    nc.vector.memset(y_im[:], 0.0)
                    for t in range(NT):
                        pt = tr_ps.tile([128, 128], BF16, tag="trps")
                        nc.tensor.transpose(pt[:], r_bf[:, t * 128:(t + 1) * 128],
                                            ident[:])
                        nc.vector.tensor_copy(z_own[:, t, 0:F], pt[:])
                        nc.vector.memset(z_own[:, t, F:F2], 0.0)
                        nc.vector.tensor_copy(y_re[:, t, :], pt[:])
                        nc.sync.dma_start(zin[t * 128:(t + 1) * 128, :],
                                          z_own[:, t, :])
                else:
                    # x2 = relu(out_acc), feature-major f32 -> DRAM
                    res = pp.tile([128, NT * F], F32, tag="res")
                    nc.vector.tensor_scalar_max(res[:], out_acc[:], 0.0)
                    nc.sync.dma_start(xfeat_out[:], res[:])

    nc.compile()
    return nc


# ----------------------------------------------------------------------------
# entry point
# ----------------------------------------------------------------------------

def kernel(x, edge_index, W_real1, Wc1, W_real2, Wc2, h, alpha,
           pool_w, lin_W, lin_b):
    x = np.asarray(x, np.float32)
    edge_index = np.asarray(edge_index)
    row, col = edge_index[0].astype(np.int64), edge_index[1].astype(np.int64)

    GT, tabs = _build_edge_tables(row, col)
    if "nc" not in _cache or _cache.get("GT") != GT:
        _cache["nc"] = _build(GT)
        _cache["GT"] = GT
    nc = _cache["nc"]

    deg = np.bincount(row, minlength=N).astype(np.float64)

    # per-node scale columns, both convs
    scal = np.zeros((NCORES, 128, 2 * NSCAL * NT), np.float32)
    for ci in range(2):
        sc = _scales(deg, float(np.asarray(h)[ci]), float(np.asarray(alpha)[ci]))
        for k, name in enumerate(SCAL_NAMES):
            cols = _shard_cols(sc[name].astype(np.float32))
            scal[:, :, (ci * NSCAL + k) * NT:(ci * NSCAL + k + 1) * NT] = cols

    # weights: lhsT layouts [cin, cout] bf16; imag pre-negated
    def T16(w):
        return np.ascontiguousarray(w.T).astype(ml_dtypes.bfloat16)
    wts = np.zeros((128, 10 * 128), ml_dtypes.bfloat16)
    packs = [T16(W_real1), T16(Wc1[0, :, :, 0]), T16(-Wc1[0, :, :, 1]),
             T16(Wc1[1, :, :, 0]), T16(-Wc1[1, :, :, 1]),
             T16(W_real2), T16(Wc2[0, :, :, 0]), T16(-Wc2[0, :, :, 1]),
             T16(Wc2[1, :, :, 0]), T16(-Wc2[1, :, :, 1])]
    for k, w in enumerate(packs):
        wts[:, k * 128:(k + 1) * 128] = w

    xn = _node_major(x)                                   # [NCORES,128,NT,F]
    xz = np.zeros((NCORES, 128, NT, F2), ml_dtypes.bfloat16)
    xz[:, :, :, :F] = xn.astype(ml_dtypes.bfloat16)
    y0 = xn.reshape(NCORES, 128, NT * F)

    ident = np.eye(128, dtype=ml_dtypes.bfloat16)

    (idx1, S1), (idx2, S2) = tabs["O1"], tabs["O2"]
    in_maps = []
    for c in range(NCORES):
        in_maps.append({
            "xz": xz[c], "y0": y0[c],
            "idx1": idx1[c], "idx2": idx2[c],
            "s1": S1[c], "s2": S2[c],
            "scal": scal[c], "wts": wts, "ident": ident,
        })

    import os
    trace = os.environ.get("KERNEL_TRACE", "0") == "1"
    res = run_bass_kernel_spmd(nc, in_maps, core_ids=list(range(NCORES)),
                               trace=trace)
    _cache["last_results"] = res

    # unshard x2: xfeat[c][o, t*128+p] -> x2[c*2500 + t*128 + p, o]
    x2 = np.empty((N, HID), np.float32)
    for c in range(NCORES):
        xf = res.results[c]["xfeat"].reshape(128, NT * F)
        x2[c * NLOC:(c + 1) * NLOC] = xf.T[:NLOC]

    # host tail: tanh score, top-k (stable ties), weighted mean, linear
    pw = np.asarray(pool_w, np.float32)
    score = np.tanh((x2 @ pw) / np.linalg.norm(pw)).astype(np.float32)
    kpool = int(np.ceil(RATIO * N))
    idx = np.argsort(-score, kind="stable")[:kpool]
    x_sel = x2[idx] * score[idx][:, None]
    pooled = x_sel.mean(axis=0, keepdims=True).astype(np.float32)
    return (pooled @ np.asarray(lin_W, np.float32).T
            + np.asarray(lin_b, np.float32)).astype(np.float32)



# revision 4
# speedup vs baseline: 1.2714x; 1.2714x over previous
"""Trainium2 Bass kernel for nn_CayleyNet (gnn_message_passing), 8 NeuronCores.

Strategy (graph/data parallel, per sharding hint):
- Nodes sharded 2500/core (padded to 2560 = 20 tiles x 128 partitions).
- Edges partitioned by scatter-destination; per destination-tile groups of
  GT x 128 edge slots (host-sorted/padded). Two orderings: O1 (scatter=row,
  gather=col; used by the B apply) and O2 (scatter=col, gather=row; Jacobi).
- CayleyNet edge weights depend only on one endpoint (tmp_left[row]), so every
  sparse op is an *unweighted* adjacency apply + per-node complex scalings:
      B y = -h*tl (.) (A1 @ y) + b_dia (.) y
      Jacobi: yk' = A2 @ (h*tl (.) yk) + b_j
- Per propagation: AllGather bf16 node-state table (re||im, 512B rows) ->
  dma_gather source rows -> one-hot S matmuls on TensorE (segment-sum into
  PSUM, f32) -> fused DVE combines (scalar_tensor_tensor with per-partition
  node scalars).
- Dense W / Wc matmuls in bf16 via PE with PE transposes between node-major
  and feature-major layouts. Small weights replicated.
- Device computes x2 (feature-major, f32). Host does tanh-score / top-k /
  weighted mean / final linear (~0.25% of FLOPs; top-k selection).
"""
import numpy as np
import ml_dtypes

import concourse.bass as bass
import concourse.bacc as bacc
import concourse.mybir as mybir
import concourse.tile as tile
from concourse.bass_utils import run_bass_kernel_spmd

# ---- problem constants (hardcoded per spec) ----
N = 20000
E = 320000
FEAT = 128
HID = 128
OUT = 10
R = 2
K = 3
RATIO = 0.9
NCORES = 8
NLOC = 2500
NT = 20                  # node tiles per core
NLOC_PAD = NT * 128      # 2560
ZROWS = NCORES * NLOC_PAD
F = 128                  # feature width
F2 = 2 * F               # re||im row width of the z table
ET = 128                 # edges per tile

BF16 = mybir.dt.bfloat16
F32 = mybir.dt.float32
I16 = mybir.dt.int16

_cache = {}


# ----------------------------------------------------------------------------
# host preprocessing
# ----------------------------------------------------------------------------

def _zrow(g):
    """z-table row for global node id g (tile-major local layout)."""
    c = g // NLOC
    return c * NLOC_PAD + (g - c * NLOC)


def _build_edge_tables(row, col):
    """Per ordering/core: gather-idx (wrapped int16) + one-hot S blocks.

    Returns (GT, tabs) where tabs[name] = (idx_wrapped [NCORES,128,cols] i16,
    S [NCORES, NT, 128, GT*128] bf16).
    """
    # determine GT uniformly: max edges landing in any (core, dst-tile) group
    maxg = 0
    for dst in (row, col):
        loc = dst % NLOC
        gid = (dst // NLOC) * NT + loc // 128   # global group id (core*NT + tile)
        cnt = np.bincount(gid, minlength=NCORES * NT)
        maxg = max(maxg, int(cnt.max()))
    GT = int(np.ceil(maxg / ET))
    ESLOTS = NT * GT * ET

    tabs = {}
    for name, dst, src in (("O1", row, col), ("O2", col, row)):
        order = np.argsort(dst, kind="stable")
        dst_s, src_s = dst[order], src[order]
        idx_all = np.zeros((NCORES, ESLOTS), np.int32)
        S_all = np.zeros((NCORES, NT * GT, ET, ET), np.float32)
        zr = _zrow(src_s)
        d_loc = dst_s % NLOC
        d_core = dst_s // NLOC
        d_tile = d_loc // 128
        d_slot = d_loc - d_tile * 128
        for c in range(NCORES):
            m = d_core == c
            dt_c, ds_c, zr_c = d_tile[m], d_slot[m], zr[m]
            for g in range(NT):
                gm = dt_c == g
                nd = int(gm.sum())
                assert nd <= GT * ET
                base = g * GT * ET
                idx_all[c, base:base + nd] = zr_c[gm]
                ts = np.arange(nd)
                S_all[c, g * GT + ts // ET, ts % ET, ds_c[gm]] = 1.0
        # wrap idx: logical i -> [i%16, i//16], replicated to 128 partitions
        w = idx_all.reshape(NCORES, ESLOTS // 16, 16).transpose(0, 2, 1)
        w = np.tile(w, (1, 8, 1)).astype(np.int16)
        # S layout for streaming: [NT, 128e, GT*128d]
        S_flat = (S_all.reshape(NCORES, NT, GT, ET, ET)
                  .transpose(0, 1, 3, 2, 4)
                  .reshape(NCORES, NT, ET, GT * ET)
                  .astype(ml_dtypes.bfloat16))
        tabs[name] = (w, S_flat)
    return GT, tabs


def _shard_cols(v):
    """[N] -> [NCORES, 128, NT] per-node columns (node (t,p) -> [:, p, t])."""
    out = np.zeros((NCORES, 128, NT), np.float32)
    pad = np.zeros(NCORES * NLOC_PAD, np.float32)
    for c in range(NCORES):
        pad[c * NLOC_PAD: c * NLOC_PAD + NLOC] = v[c * NLOC: (c + 1) * NLOC]
    lp = pad.reshape(NCORES, NT, 128)
    return lp.transpose(0, 2, 1).copy()


def _node_major(x):
    """[N, F] f32 -> [NCORES, 128, NT, F]: node (t,p) at [c, p, t, :]."""
    out = np.zeros((NCORES, NT, 128, x.shape[1]), np.float32)
    for c in range(NCORES):
        out[c].reshape(NLOC_PAD, -1)[:NLOC] = x[c * NLOC:(c + 1) * NLOC]
    return out.transpose(0, 2, 1, 3).copy()


def _scales(deg, h, alpha):
    """Per-conv per-node scale columns. Returns dict of [N] f32 arrays.

    s = B post-scale, d = b_dia, g = Jacobi pre-scale; gs = g*s and gd = g*d
    fold the first Jacobi gather operand z = g.(s.u + d.y) into one chain.
    """
    l = (deg - alpha).astype(np.float64)
    tl = 1.0 / (h * l + 1j)
    s = -h * tl
    d = tl * (h * l - 1j)
    g = h * tl
    gs = g * s
    gd = g * d
    out = {}
    for nm, v in (("s", s), ("d", d), ("g", g), ("gs", gs), ("gd", gd)):
        out[nm + "_re"] = np.real(v)
        out[nm + "_im"] = np.imag(v)
        out["n" + nm + "_im"] = -np.imag(v)
    return out


SCAL_NAMES = ["s_re", "s_im", "ns_im", "d_re", "d_im", "nd_im",
              "g_re", "g_im", "ng_im",
              "gs_re", "gs_im", "ngs_im", "gd_re", "gd_im", "ngd_im"]
NSCAL = len(SCAL_NAMES)


# ----------------------------------------------------------------------------
# kernel builder
# ----------------------------------------------------------------------------

def _build(GT):
    ESLOTS = NT * GT * ET
    ICOLS = ESLOTS // 16
    GCOLS = GT * ET // 16     # idx cols per group

    nc = bacc.Bacc("TRN2", target_bir_lowering=False, debug=False,
                   num_devices=NCORES, num_swdge_queues=4)

    xz_in = nc.dram_tensor("xz", [128, NT, F2], BF16, kind="ExternalInput")
    y0_in = nc.dram_tensor("y0", [128, NT * F], F32, kind="ExternalInput")
    idx1_in = nc.dram_tensor("idx1", [128, ICOLS], I16, kind="ExternalInput")
    idx2_in = nc.dram_tensor("idx2", [128, ICOLS], I16, kind="ExternalInput")
    s1_in = nc.dram_tensor("s1", [NT, 128, GT * ET], BF16, kind="ExternalInput")
    s2_in = nc.dram_tensor("s2", [NT, 128, GT * ET], BF16, kind="ExternalInput")
    scal_in = nc.dram_tensor("scal", [128, 2 * NSCAL * NT], F32, kind="ExternalInput")
    wts_in = nc.dram_tensor("wts", [128, 10 * 128], BF16, kind="ExternalInput")
    ident_in = nc.dram_tensor("ident", [128, 128], BF16, kind="ExternalInput")
    xfeat_out = nc.dram_tensor("xfeat", [128, NT * F], F32, kind="ExternalOutput")

    s_dram = {"O1": s1_in, "O2": s2_in}

    with tile.TileContext(nc) as tc:
        with (
            tc.tile_pool(name="persist", bufs=1) as pp,
            tc.tile_pool(name="gpool", bufs=4) as gpool,
            tc.tile_pool(name="spool", bufs=8) as spool,
            tc.tile_pool(name="tmp", bufs=4) as tmpp,
            tc.tile_pool(name="prop_ps", bufs=5, space="PSUM") as prop_ps,
            tc.tile_pool(name="tr_ps", bufs=2, space="PSUM") as tr_ps,
            tc.tile_pool(name="mm_ps", bufs=1, space="PSUM") as mm_ps,
            tc.tile_pool(name="dram", bufs=1, space="DRAM") as dram,
        ):
            # ---- persistent SBUF state ----
            z_own = pp.tile([128, NT, F2], BF16)        # bf16 node state (re||im)
            y_re = pp.tile([128, NT, F], F32)
            y_im = pp.tile([128, NT, F], F32)
            b_re = pp.tile([128, NT, F], F32)
            b_im = pp.tile([128, NT, F], F32)
            out_acc = pp.tile([128, NT * F], F32)       # feature-major conv accum
            xT = pp.tile([128, NT * F], BF16)           # transposed input [c, n]
            yT_re = pp.tile([128, NT * F], BF16)
            yT_im = pp.tile([128, NT * F], BF16)
            idx_sb = {"O1": pp.tile([128, ICOLS], I16, name="idx_o1"),
                      "O2": pp.tile([128, ICOLS], I16, name="idx_o2")}
            scal_sb = pp.tile([128, 2 * NSCAL * NT], F32)
            wts_sb = pp.tile([128, 10 * 128], BF16)
            ident = pp.tile([128, 128], BF16)

            zin = dram.tile([NLOC_PAD, F2], BF16)
            # one Shared AllGather output per propagation (single-writer rule)
            ztabs = [dram.tile([ZROWS, F2], BF16, addr_space="Shared",
                               name=f"ztab{i}")
                     for i in range(2 * R * (1 + K))]
            prop_counter = [0]

            # ---- load constants ----
            nc.sync.dma_start(idx_sb["O1"][:], idx1_in[:])
            nc.sync.dma_start(idx_sb["O2"][:], idx2_in[:])
            nc.sync.dma_start(scal_sb[:], scal_in[:])
            nc.sync.dma_start(wts_sb[:], wts_in[:])
            nc.sync.dma_start(ident[:], ident_in[:])
            nc.sync.dma_start(z_own[:], xz_in[:])
            nc.sync.dma_start(y_re[:], y0_in[:])
            nc.vector.memset(y_im[:], 0.0)
            for t in range(NT):
                nc.sync.dma_start(zin[t * 128:(t + 1) * 128, :], z_own[:, t, :])

            def col(ci, name, t):
                k = ci * NSCAL + SCAL_NAMES.index(name)
                return scal_sb[:, k * NT + t: k * NT + t + 1]

            def wt(k):
                return wts_sb[:, k * 128:(k + 1) * 128]

            def prop(ordering, consumer):
                """One adjacency apply: AllGather z -> gather -> S matmuls.

                zin rows are written per-tile by the PREVIOUS prop's consumers
                (via zin_tile below), so only the AllGather remains here.
                """
                ztab = ztabs[prop_counter[0]]
                prop_counter[0] += 1
                nc.gpsimd.collective_compute(
                    "AllGather", mybir.AluOpType.bypass,
                    replica_groups=[list(range(NCORES))],
                    ins=[zin.opt()], outs=[ztab.opt()],
                )
                for g in range(NT):
                    gb = gpool.tile([128, GT, F2], BF16, tag="gbuf")
                    nc.gpsimd.dma_gather(
                        gb[:], ztab[:],
                        idx_sb[ordering][:, g * GCOLS:(g + 1) * GCOLS],
                        num_idxs=GT * ET, num_idxs_reg=GT * ET, elem_size=F2,
                        single_packet=False, queue_num=g % 4,
                    )
                    ssb = spool.tile([128, GT * ET], BF16, tag="schunk")
                    nc.sync.dma_start(ssb[:], s_dram[ordering][g])
                    ps = prop_ps.tile([128, F2], F32, tag="prop_ps")
                    for t in range(GT):
                        nc.tensor.matmul(ps[:], ssb[:, t * ET:(t + 1) * ET],
                                         gb[:, t, :],
                                         start=(t == 0), stop=(t == GT - 1))
                    consumer(g, ps)

            STT = nc.vector.scalar_tensor_tensor
            MUL = mybir.AluOpType.mult
            ADD = mybir.AluOpType.add
            COPY = mybir.ActivationFunctionType.Copy

            def smul(out_ap, in_ap, c_ap):
                """out = in * per-partition scalar, on the (idle) Scalar engine."""
                nc.scalar.activation(out_ap, in_ap, COPY, scale=c_ap)

            def zin_tile(g):
                """Push this dst tile's fresh z rows to the DRAM table input."""
                nc.sync.dma_start(zin[g * 128:(g + 1) * 128, :], z_own[:, g, :])

            def b_consumer(ci):
                def consume(g, ps):
                    u_re, u_im = ps[:, 0:F], ps[:, F:F2]
                    # critical path first: z = gs.u + gd.y (both complex prods)
                    tmp = tmpp.tile([128, F], F32, tag="ctmp")
                    smul(tmp[:], u_re, col(ci, "gs_re", g))
                    STT(tmp[:], u_im, col(ci, "ngs_im", g), tmp[:], MUL, ADD)
                    STT(tmp[:], y_re[:, g, :], col(ci, "gd_re", g), tmp[:], MUL, ADD)
                    STT(z_own[:, g, 0:F], y_im[:, g, :], col(ci, "ngd_im", g), tmp[:], MUL, ADD)
                    tmp2 = tmpp.tile([128, F], F32, tag="ctmp2")
                    smul(tmp2[:], u_im, col(ci, "gs_re", g))
                    STT(tmp2[:], u_re, col(ci, "gs_im", g), tmp2[:], MUL, ADD)
                    STT(tmp2[:], y_im[:, g, :], col(ci, "gd_re", g), tmp2[:], MUL, ADD)
                    STT(z_own[:, g, F:F2], y_re[:, g, :], col(ci, "gd_im", g), tmp2[:], MUL, ADD)
                    zin_tile(g)
                    # deferred: b = s.u + d.y (consumed by next prop's combines)
                    tmp3 = tmpp.tile([128, F], F32, tag="ctmp3")
                    smul(tmp3[:], u_re, col(ci, "s_re", g))
                    STT(tmp3[:], u_im, col(ci, "ns_im", g), tmp3[:], MUL, ADD)
                    STT(tmp3[:], y_re[:, g, :], col(ci, "d_re", g), tmp3[:], MUL, ADD)
                    STT(b_re[:, g, :], y_im[:, g, :], col(ci, "nd_im", g), tmp3[:], MUL, ADD)
                    tmp4 = tmpp.tile([128, F], F32, tag="ctmp4")
                    smul(tmp4[:], u_im, col(ci, "s_re", g))
                    STT(tmp4[:], u_re, col(ci, "s_im", g), tmp4[:], MUL, ADD)
                    STT(tmp4[:], y_im[:, g, :], col(ci, "d_re", g), tmp4[:], MUL, ADD)
                    STT(b_im[:, g, :], y_re[:, g, :], col(ci, "d_im", g), tmp4[:], MUL, ADD)
                return consume

            def jacobi_consumer(ci, last):
                def consume(g, ps):
                    u_re, u_im = ps[:, 0:F], ps[:, F:F2]
                    # y = u + b   (this is yk)
                    nc.vector.tensor_tensor(y_re[:, g, :], u_re, b_re[:, g, :], ADD)
                    nc.vector.tensor_tensor(y_im[:, g, :], u_im, b_im[:, g, :], ADD)
                    if last:
                        # z = bf16(y) for next B apply / Wc transposes
                        smul(z_own[:, g, 0:F], y_re[:, g, :], 1.0)
                        smul(z_own[:, g, F:F2], y_im[:, g, :], 1.0)
                    else:
                        # z = g (.) y   (next Jacobi gather operand)
                        tmp = tmpp.tile([128, F], F32, tag="ctmp")
                        smul(tmp[:], y_re[:, g, :], col(ci, "g_re", g))
                        STT(z_own[:, g, 0:F], y_im[:, g, :], col(ci, "ng_im", g), tmp[:], MUL, ADD)
                        tmp2 = tmpp.tile([128, F], F32, tag="ctmp2")
                        smul(tmp2[:], y_im[:, g, :], col(ci, "g_re", g))
                        STT(z_own[:, g, F:F2], y_re[:, g, :], col(ci, "g_im", g), tmp2[:], MUL, ADD)
                    zin_tile(g)
                return consume

            def transpose_to(dst, src_ap, t):
                """dst[:, t*128:(t+1)*128] = src_ap.T (both bf16)."""
                pt = tr_ps.tile([128, 128], BF16, tag="trps")
                nc.tensor.transpose(pt[:], src_ap, ident[:])
                nc.vector.tensor_copy(dst[:, t * 128:(t + 1) * 128], pt[:])

            def dense_chunks(lhs_ks, rhs_list, first):
                """out_acc[:, ch] (+)= sum_i lhsT(k_i) @ rhs_i[:, ch] (x2 if not first)."""
                nch = NT * F // 512
                for ch in range(nch):
                    sl = slice(ch * 512, (ch + 1) * 512)
                    ps = mm_ps.tile([128, 512], F32, tag="mmps")
                    for i, (k, rhs) in enumerate(zip(lhs_ks, rhs_list)):
                        nc.tensor.matmul(ps[:], wt(k), rhs[:, sl],
                                         start=(i == 0), stop=(i == len(lhs_ks) - 1))
                    if first:
                        nc.vector.tensor_copy(out_acc[:, sl], ps[:])
                    else:
                        STT(out_acc[:, sl], ps[:], 2.0, out_acc[:, sl], MUL, ADD)

            # ================= conv block =================
            for ci in range(2):
                wbase = ci * 5
                # xT = transpose(x_bf) from z re-halves
                for t in range(NT):
                    transpose_to(xT, z_own[:, t, 0:F], t)
                dense_chunks([wbase + 0], [xT], first=True)

                for j in range(R):
                    prop("O1", b_consumer(ci))
                    for it in range(K):
                        prop("O2", jacobi_consumer(ci, last=(it == K - 1)))
                    # yT from z halves (bf16 copies of y)
                    for t in range(NT):
                        transpose_to(yT_re, z_own[:, t, 0:F], t)
                        transpose_to(yT_im, z_own[:, t, F:F2], t)
                    dense_chunks([wbase + 1 + 2 * j, wbase + 2 + 2 * j],
                                 [yT_re, yT_im], first=False)

                if ci == 0:
                    # relu -> bf16, transpose back to node-major, reseed state
                    r_bf = pp.tile([128, NT * F], BF16, tag="rbf")
                    nc.vector.tensor_scalar_max(r_bf[:], out_acc[:], 0.0)
                    nc.vector.memset(y_im[:], 0.0)
                    for t in range(NT):
                        pt = tr_ps.tile([128, 128], BF16, tag="trps")
                        nc.tensor.transpose(pt[:], r_bf[:, t * 128:(t + 1) * 128],
                                            ident[:])
                        nc.vector.tensor_copy(z_own[:, t, 0:F], pt[:])
                        nc.vector.memset(z_own[:, t, F:F2], 0.0)
                        nc.vector.tensor_copy(y_re[:, t, :], pt[:])
                        nc.sync.dma_start(zin[t * 128:(t + 1) * 128, :],
                                          z_own[:, t, :])
                else:
                    # x2 = relu(out_acc), feature-major f32 -> DRAM
                    res = pp.tile([128, NT * F], F32, tag="res")
                    nc.vector.tensor_scalar_max(res[:], out_acc[:], 0.0)
                    nc.sync.dma_start(xfeat_out[:], res[:])

    nc.compile()
    return nc


# ----------------------------------------------------------------------------
# entry point
# ----------------------------------------------------------------------------

def kernel(x, edge_index, W_real1, Wc1, W_real2, Wc2, h, alpha,
           pool_w, lin_W, lin_b):
    x = np.asarray(x, np.float32)
    edge_index = np.asarray(edge_index)
    row, col = edge_index[0].astype(np.int64), edge_index[1].astype(np.int64)

    GT, tabs = _build_edge_tables(row, col)
    if "nc" not in _cache or _cache.get("GT") != GT:
        _cache["nc"] = _build(GT)
        _cache["GT"] = GT
    nc = _cache["nc"]

    deg = np.bincount(row, minlength=N).astype(np.float64)

    # per-node scale columns, both convs
    scal = np.zeros((NCORES, 128, 2 * NSCAL * NT), np.float32)
    for ci in range(2):
        sc = _scales(deg, float(np.asarray(h)[ci]), float(np.asarray(alpha)[ci]))
        for k, name in enumerate(SCAL_NAMES):
            cols = _shard_cols(sc[name].astype(np.float32))
            scal[:, :, (ci * NSCAL + k) * NT:(ci * NSCAL + k + 1) * NT] = cols

    # weights: lhsT layouts [cin, cout] bf16; imag pre-negated
    def T16(w):
        return np.ascontiguousarray(w.T).astype(ml_dtypes.bfloat16)
    wts = np.zeros((128, 10 * 128), ml_dtypes.bfloat16)
    packs = [T16(W_real1), T16(Wc1[0, :, :, 0]), T16(-Wc1[0, :, :, 1]),
             T16(Wc1[1, :, :, 0]), T16(-Wc1[1, :, :, 1]),
             T16(W_real2), T16(Wc2[0, :, :, 0]), T16(-Wc2[0, :, :, 1]),
             T16(Wc2[1, :, :, 0]), T16(-Wc2[1, :, :, 1])]
    for k, w in enumerate(packs):
        wts[:, k * 128:(k + 1) * 128] = w

    xn = _node_major(x)                                   # [NCORES,128,NT,F]
    xz = np.zeros((NCORES, 128, NT, F2), ml_dtypes.bfloat16)
    xz[:, :, :, :F] = xn.astype(ml_dtypes.bfloat16)
    y0 = xn.reshape(NCORES, 128, NT * F)

    ident = np.eye(128, dtype=ml_dtypes.bfloat16)

    (idx1, S1), (idx2, S2) = tabs["O1"], tabs["O2"]
    in_maps = []
    for c in range(NCORES):
        in_maps.append({
            "xz": xz[c], "y0": y0[c],
            "idx1": idx1[c], "idx2": idx2[c],
            "s1": S1[c], "s2": S2[c],
            "scal": scal[c], "wts": wts, "ident": ident,
        })

    import os
    trace = os.environ.get("KERNEL_TRACE", "0") == "1"
    res = run_bass_kernel_spmd(nc, in_maps, core_ids=list(range(NCORES)),
                               trace=trace)
    _cache["last_results"] = res

    # unshard x2: xfeat[c][o, t*128+p] -> x2[c*2500 + t*128 + p, o]
    x2 = np.empty((N, HID), np.float32)
    for c in range(NCORES):
        xf = res.results[c]["xfeat"].reshape(128, NT * F)
        x2[c * NLOC:(c + 1) * NLOC] = xf.T[:NLOC]

    # host tail: tanh score, top-k (stable ties), weighted mean, linear
    pw = np.asarray(pool_w, np.float32)
    score = np.tanh((x2 @ pw) / np.linalg.norm(pw)).astype(np.float32)
    kpool = int(np.ceil(RATIO * N))
    idx = np.argsort(-score, kind="stable")[:kpool]
    x_sel = x2[idx] * score[idx][:, None]
    pooled = x_sel.mean(axis=0, keepdims=True).astype(np.float32)
    return (pooled @ np.asarray(lin_W, np.float32).T
            + np.asarray(lin_b, np.float32)).astype(np.float32)



# revision 18
# speedup vs baseline: 1.9548x; 1.5374x over previous
"""Trainium2 Bass kernel for nn_CayleyNet (gnn_message_passing), 8 NeuronCores.

Strategy (graph/data parallel, per sharding hint):
- Nodes sharded 2500/core (padded to 2560 = 20 tiles x 128 partitions).
- Edges partitioned by scatter-destination; per destination-tile groups of
  GT x 128 edge slots (host-sorted/padded). Two orderings: O1 (scatter=row,
  gather=col; used by the B apply) and O2 (scatter=col, gather=row; Jacobi).
- CayleyNet edge weights depend only on one endpoint (tmp_left[row]), so every
  sparse op is an *unweighted* adjacency apply + per-node complex scalings:
      B y = -h*tl (.) (A1 @ y) + b_dia (.) y
      Jacobi: yk' = A2 @ (h*tl (.) yk) + b_j
- Per propagation: AllGather bf16 node-state table (re||im, 512B rows) ->
  dma_gather source rows -> one-hot S matmuls on TensorE (segment-sum into
  PSUM, f32) -> fused DVE combines (scalar_tensor_tensor with per-partition
  node scalars).
- Dense W / Wc matmuls in bf16 via PE with PE transposes between node-major
  and feature-major layouts. Small weights replicated.
- Device computes x2 (feature-major, f32). Host does tanh-score / top-k /
  weighted mean / final linear (~0.25% of FLOPs; top-k selection).
"""
import numpy as np
import ml_dtypes

import concourse.bass as bass
import concourse.bacc as bacc
import concourse.mybir as mybir
import concourse.tile as tile
from concourse.bass_utils import run_bass_kernel_spmd

# ---- problem constants (hardcoded per spec) ----
N = 20000
E = 320000
FEAT = 128
HID = 128
OUT = 10
R = 2
K = 3
RATIO = 0.9
NCORES = 8
NLOC = 2500
NT = 20                  # node tiles per core
NLOC_PAD = NT * 128      # 2560
ZROWS = NCORES * NLOC_PAD
F = 128                  # feature width
F2 = 2 * F               # re||im row width of the z table
ET = 128                 # edges per tile

BF16 = mybir.dt.bfloat16
F32 = mybir.dt.float32
I16 = mybir.dt.int16

_cache = {}


# ----------------------------------------------------------------------------
# host preprocessing
# ----------------------------------------------------------------------------

def _assign_nodes(row, col):
    """Balance nodes into NCORES*NT buckets of <=128 nodes each.

    Greedy LPT on max(out-edge, in-edge) bucket load so that the per-bucket
    edge counts of BOTH orderings (O1 groups by row, O2 groups by col) stay
    close to the E/(NCORES*NT)=2000 average; the padded group size GT then
    drops to its floor of ceil(2000/128)=16 tiles instead of tracking the
    max of an unbalanced contiguous split.
    Returns (node_core, node_tile, node_slot) arrays of length N.
    """
    rowdeg = np.bincount(row, minlength=N).astype(np.int64)
    coldeg = np.bincount(col, minlength=N).astype(np.int64)
    nb = NCORES * NT
    rsum = np.zeros(nb)
    csum = np.zeros(nb)
    cnt = np.zeros(nb, np.int64)
    order = np.argsort(-(rowdeg + coldeg), kind="stable")
    node_bucket = np.empty(N, np.int64)
    node_slot = np.empty(N, np.int64)
    for g in order:
        load = np.maximum(rsum + rowdeg[g], csum + coldeg[g])
        load[cnt >= 128] = np.inf
        b = int(np.argmin(load))
        node_bucket[g] = b
        node_slot[g] = cnt[b]
        rsum[b] += rowdeg[g]
        csum[b] += coldeg[g]
        cnt[b] += 1
    return node_bucket // NT, node_bucket % NT, node_slot


def _build_edge_tables(row, col, node_core, node_tile, node_slot):
    """Per ordering/core: gather-idx (wrapped int16) + one-hot S blocks.

    Returns (GT, tabs) where tabs[name] = (idx_wrapped [NCORES,128,cols] i16,
    S [NCORES, NT, 128, GT*128] bf16).
    """
    # determine GT uniformly: max edges landing in any (core, dst-tile) group
    zrow_arr = node_core * NLOC_PAD + node_tile * 128 + node_slot
    bucket = node_core * NT + node_tile
    maxg = 0
    for dst in (row, col):
        cnt = np.bincount(bucket[dst], minlength=NCORES * NT)
        maxg = max(maxg, int(cnt.max()))
    GT = int(np.ceil(maxg / ET))
    ESLOTS = NT * GT * ET

    tabs = {}
    for name, dst, src in (("O1", row, col), ("O2", col, row)):
        order = np.argsort(bucket[dst], kind="stable")
        dst_s, src_s = dst[order], src[order]
        idx_all = np.zeros((NCORES, ESLOTS), np.int32)
        S_all = np.zeros((NCORES, NT * GT, ET, ET), np.float32)
        zr = zrow_arr[src_s]
        d_core = node_core[dst_s]
        d_tile = node_tile[dst_s]
        d_slot = node_slot[dst_s]
        for c in range(NCORES):
            m = d_core == c
            dt_c, ds_c, zr_c = d_tile[m], d_slot[m], zr[m]
            for g in range(NT):
                gm = dt_c == g
                nd = int(gm.sum())
                assert nd <= GT * ET
                base = g * GT * ET
                idx_all[c, base:base + nd] = zr_c[gm]
                ts = np.arange(nd)
                S_all[c, g * GT + ts // ET, ts % ET, ds_c[gm]] = 1.0
        # wrap idx: logical i -> [i%16, i//16], replicated to 128 partitions
        w = idx_all.reshape(NCORES, ESLOTS // 16, 16).transpose(0, 2, 1)
        w = np.tile(w, (1, 8, 1)).astype(np.int16)
        # S layout for streaming: [NT, 128e, GT*128d]
        S_flat = (S_all.reshape(NCORES, NT, GT, ET, ET)
                  .transpose(0, 1, 3, 2, 4)
                  .reshape(NCORES, NT, ET, GT * ET)
                  .astype(ml_dtypes.bfloat16))
        tabs[name] = (w, S_flat)
    return GT, tabs


def _shard_cols(v, node_core, node_tile, node_slot):
    """[N] -> [NCORES, 128, NT] per-node columns (node (c,t,p) -> [c, p, t])."""
    out = np.zeros((NCORES, 128, NT), np.float32)
    out[node_core, node_slot, node_tile] = v
    return out


def _node_major(x, node_core, node_tile, node_slot):
    """[N, F] f32 -> [NCORES, 128, NT, F]: node (c,t,p) at [c, p, t, :]."""
    out = np.zeros((NCORES, 128, NT, x.shape[1]), np.float32)
    out[node_core, node_slot, node_tile] = x
    return out


def _scales(deg, h, alpha):
    """Per-conv per-node scale columns. Returns dict of [N] f32 arrays.

    s = B post-scale, d = b_dia, g = Jacobi pre-scale; gs = g*s and gd = g*d
    fold the first Jacobi gather operand z = g.(s.u + d.y) into one chain.
    """
    l = (deg - alpha).astype(np.float64)
    tl = 1.0 / (h * l + 1j)
    s = -h * tl
    d = tl * (h * l - 1j)
    g = h * tl
    gs = g * s
    gd = g * d
    out = {}
    for nm, v in (("s", s), ("d", d), ("g", g), ("gs", gs), ("gd", gd)):
        out[nm + "_re"] = np.real(v)
        out[nm + "_im"] = np.imag(v)
        out["n" + nm + "_im"] = -np.imag(v)
    return out


SCAL_NAMES = ["s_re", "s_im", "ns_im", "d_re", "d_im", "nd_im",
              "g_re", "g_im", "ng_im",
              "gs_re", "gs_im", "ngs_im", "gd_re", "gd_im", "ngd_im"]
NSCAL = len(SCAL_NAMES)


# ----------------------------------------------------------------------------
# kernel builder
# ----------------------------------------------------------------------------

def _build(GT):
    ESLOTS = NT * GT * ET
    ICOLS = ESLOTS // 16
    GCOLS = GT * ET // 16     # idx cols per group

    nc = bacc.Bacc("TRN2", target_bir_lowering=False, debug=False,
                   num_devices=NCORES, num_swdge_queues=4)

    xz_in = nc.dram_tensor("xz", [128, NT, F2], BF16, kind="ExternalInput")
    y0_in = nc.dram_tensor("y0", [128, NT * F], F32, kind="ExternalInput")
    idx1_in = nc.dram_tensor("idx1", [128, ICOLS], I16, kind="ExternalInput")
    idx2_in = nc.dram_tensor("idx2", [128, ICOLS], I16, kind="ExternalInput")
    s1_in = nc.dram_tensor("s1", [NT, 128, GT * ET], BF16, kind="ExternalInput")
    s2_in = nc.dram_tensor("s2", [NT, 128, GT * ET], BF16, kind="ExternalInput")
    scal_in = nc.dram_tensor("scal", [128, 2 * NSCAL * NT], F32, kind="ExternalInput")
    wts_in = nc.dram_tensor("wts", [128, 10 * 128], BF16, kind="ExternalInput")
    ident_in = nc.dram_tensor("ident", [128, 128], BF16, kind="ExternalInput")
    xfeat_out = nc.dram_tensor("xfeat", [128, NT * F], F32, kind="ExternalOutput")

    s_dram = {"O1": s1_in, "O2": s2_in}

    with tile.TileContext(nc) as tc:
        with (
            tc.tile_pool(name="persist", bufs=1) as pp,
            tc.tile_pool(name="gpool", bufs=4) as gpool,
            tc.tile_pool(name="spool", bufs=8) as spool,
            tc.tile_pool(name="tmp", bufs=4) as tmpp,
            tc.tile_pool(name="prop_ps", bufs=5, space="PSUM") as prop_ps,
            tc.tile_pool(name="tr_ps", bufs=2, space="PSUM") as tr_ps,
            tc.tile_pool(name="mm_ps", bufs=1, space="PSUM") as mm_ps,
            tc.tile_pool(name="dram", bufs=1, space="DRAM") as dram,
        ):
            # ---- persistent SBUF state ----
            z_own = pp.tile([128, NT, F2], BF16)        # bf16 node state (re||im)
            y_re = pp.tile([128, NT, F], F32)
            y_im = pp.tile([128, NT, F], F32)
            b_re = pp.tile([128, NT, F], F32)
            b_im = pp.tile([128, NT, F], F32)
            out_acc = pp.tile([128, NT * F], F32)       # feature-major conv accum
            xT = pp.tile([128, NT * F], BF16)           # transposed input [c, n]
            yT_re = pp.tile([128, NT * F], BF16)
            yT_im = pp.tile([128, NT * F], BF16)
            idx_sb = {"O1": pp.tile([128, ICOLS], I16, name="idx_o1"),
                      "O2": pp.tile([128, ICOLS], I16, name="idx_o2")}
            scal_sb = pp.tile([128, 2 * NSCAL * NT], F32)
            wts_sb = pp.tile([128, 10 * 128], BF16)
            ident = pp.tile([128, 128], BF16)

            zin = dram.tile([NLOC_PAD, F2], BF16)
            # one Shared AllGather output per propagation (single-writer rule)
            ztabs = [dram.tile([ZROWS, F2], BF16, addr_space="Shared",
                               name=f"ztab{i}")
                     for i in range(2 * R * (1 + K))]
            prop_counter = [0]

            # ---- load constants ----
            nc.sync.dma_start(idx_sb["O1"][:], idx1_in[:])
            nc.sync.dma_start(idx_sb["O2"][:], idx2_in[:])
            nc.sync.dma_start(scal_sb[:], scal_in[:])
            nc.sync.dma_start(wts_sb[:], wts_in[:])
            nc.sync.dma_start(ident[:], ident_in[:])
            nc.sync.dma_start(z_own[:], xz_in[:])
            nc.sync.dma_start(y_re[:], y0_in[:])
            nc.vector.memset(y_im[:], 0.0)
            for t in range(NT):
                nc.sync.dma_start(zin[t * 128:(t + 1) * 128, :], z_own[:, t, :])

            def col(ci, name, t):
                k = ci * NSCAL + SCAL_NAMES.index(name)
                return scal_sb[:, k * NT + t: k * NT + t + 1]

            def wt(k):
                return wts_sb[:, k * 128:(k + 1) * 128]

            def prop(ordering, consumer):
                """One adjacency apply: AllGather z -> gather -> S matmuls.

                zin rows are written per-tile by the PREVIOUS prop's consumers
                (via zin_tile below), so only the AllGather remains here.
                """
                ztab = ztabs[prop_counter[0]]
                prop_counter[0] += 1
                nc.gpsimd.collective_compute(
                    "AllGather", mybir.AluOpType.bypass,
                    replica_groups=[list(range(NCORES))],
                    ins=[zin.opt()], outs=[ztab.opt()],
                )
                for g in range(NT):
                    gb = gpool.tile([128, GT, F2], BF16, tag="gbuf")
                    nc.gpsimd.dma_gather(
                        gb[:], ztab[:],
                        idx_sb[ordering][:, g * GCOLS:(g + 1) * GCOLS],
                        num_idxs=GT * ET, num_idxs_reg=GT * ET, elem_size=F2,
                        single_packet=False, queue_num=g % 4,
                    )
                    ssb = spool.tile([128, GT * ET], BF16, tag="schunk")
                    nc.sync.dma_start(ssb[:], s_dram[ordering][g])
                    ps = prop_ps.tile([128, F2], F32, tag="prop_ps")
                    for t in range(GT):
                        nc.tensor.matmul(ps[:], ssb[:, t * ET:(t + 1) * ET],
                                         gb[:, t, :],
                                         start=(t == 0), stop=(t == GT - 1))
                    consumer(g, ps)

            STT = nc.vector.scalar_tensor_tensor
            MUL = mybir.AluOpType.mult
            ADD = mybir.AluOpType.add
            COPY = mybir.ActivationFunctionType.Copy

            def smul(out_ap, in_ap, c_ap):
                """out = in * per-partition scalar, on the (idle) Scalar engine."""
                nc.scalar.activation(out_ap, in_ap, COPY, scale=c_ap)

            def zin_tile(g):
                """Push this dst tile's fresh z rows to the DRAM table input."""
                nc.sync.dma_start(zin[g * 128:(g + 1) * 128, :], z_own[:, g, :])

            def b_consumer(ci):
                def consume(g, ps):
                    u_re, u_im = ps[:, 0:F], ps[:, F:F2]
                    # critical path first: z = gs.u + gd.y (both complex prods)
                    tmp = tmpp.tile([128, F], F32, tag="ctmp")
                    smul(tmp[:], u_re, col(ci, "gs_re", g))
                    STT(tmp[:], u_im, col(ci, "ngs_im", g), tmp[:], MUL, ADD)
                    STT(tmp[:], y_re[:, g, :], col(ci, "gd_re", g), tmp[:], MUL, ADD)
                    STT(z_own[:, g, 0:F], y_im[:, g, :], col(ci, "ngd_im", g), tmp[:], MUL, ADD)
                    tmp2 = tmpp.tile([128, F], F32, tag="ctmp2")
                    smul(tmp2[:], u_im, col(ci, "gs_re", g))
                    STT(tmp2[:], u_re, col(ci, "gs_im", g), tmp2[:], MUL, ADD)
                    STT(tmp2[:], y_im[:, g, :], col(ci, "gd_re", g), tmp2[:], MUL, ADD)
                    STT(z_own[:, g, F:F2], y_re[:, g, :], col(ci, "gd_im", g), tmp2[:], MUL, ADD)
                    zin_tile(g)
                    # deferred: b = s.u + d.y (consumed by next prop's combines)
                    tmp3 = tmpp.tile([128, F], F32, tag="ctmp3")
                    smul(tmp3[:], u_re, col(ci, "s_re", g))
                    STT(tmp3[:], u_im, col(ci, "ns_im", g), tmp3[:], MUL, ADD)
                    STT(tmp3[:], y_re[:, g, :], col(ci, "d_re", g), tmp3[:], MUL, ADD)
                    STT(b_re[:, g, :], y_im[:, g, :], col(ci, "nd_im", g), tmp3[:], MUL, ADD)
                    tmp4 = tmpp.tile([128, F], F32, tag="ctmp4")
                    smul(tmp4[:], u_im, col(ci, "s_re", g))
                    STT(tmp4[:], u_re, col(ci, "s_im", g), tmp4[:], MUL, ADD)
                    STT(tmp4[:], y_im[:, g, :], col(ci, "d_re", g), tmp4[:], MUL, ADD)
                    STT(b_im[:, g, :], y_re[:, g, :], col(ci, "d_im", g), tmp4[:], MUL, ADD)
                return consume

            def jacobi_consumer(ci, last):
                def consume(g, ps):
                    u_re, u_im = ps[:, 0:F], ps[:, F:F2]
                    # y = u + b   (this is yk)
                    nc.vector.tensor_tensor(y_re[:, g, :], u_re, b_re[:, g, :], ADD)
                    nc.vector.tensor_tensor(y_im[:, g, :], u_im, b_im[:, g, :], ADD)
                    if last:
                        # z = bf16(y) for next B apply / Wc transposes
                        smul(z_own[:, g, 0:F], y_re[:, g, :], 1.0)
                        smul(z_own[:, g, F:F2], y_im[:, g, :], 1.0)
                    else:
                        # z = g (.) y   (next Jacobi gather operand)
                        tmp = tmpp.tile([128, F], F32, tag="ctmp")
                        smul(tmp[:], y_re[:, g, :], col(ci, "g_re", g))
                        STT(z_own[:, g, 0:F], y_im[:, g, :], col(ci, "ng_im", g), tmp[:], MUL, ADD)
                        tmp2 = tmpp.tile([128, F], F32, tag="ctmp2")
                        smul(tmp2[:], y_im[:, g, :], col(ci, "g_re", g))
                        STT(z_own[:, g, F:F2], y_re[:, g, :], col(ci, "g_im", g), tmp2[:], MUL, ADD)
                    zin_tile(g)
                return consume

            def transpose_to(dst, src_ap, t):
                """dst[:, t*128:(t+1)*128] = src_ap.T (both bf16)."""
                pt = tr_ps.tile([128, 128], BF16, tag="trps")
                nc.tensor.transpose(pt[:], src_ap, ident[:])
                nc.vector.tensor_copy(dst[:, t * 128:(t + 1) * 128], pt[:])

            def dense_chunks(lhs_ks, rhs_list, first):
                """out_acc[:, ch] (+)= sum_i lhsT(k_i) @ rhs_i[:, ch] (x2 if not first)."""
                nch = NT * F // 512
                for ch in range(nch):
                    sl = slice(ch * 512, (ch + 1) * 512)
                    ps = mm_ps.tile([128, 512], F32, tag="mmps")
                    for i, (k, rhs) in enumerate(zip(lhs_ks, rhs_list)):
                        nc.tensor.matmul(ps[:], wt(k), rhs[:, sl],
                                         start=(i == 0), stop=(i == len(lhs_ks) - 1))
                    if first:
                        nc.vector.tensor_copy(out_acc[:, sl], ps[:])
                    else:
                        STT(out_acc[:, sl], ps[:], 2.0, out_acc[:, sl], MUL, ADD)

            # ================= conv block =================
            for ci in range(2):
                wbase = ci * 5
                # xT = transpose(x_bf) from z re-halves
                for t in range(NT):
                    transpose_to(xT, z_own[:, t, 0:F], t)
                dense_chunks([wbase + 0], [xT], first=True)

                for j in range(R):
                    prop("O1", b_consumer(ci))
                    for it in range(K):
                        prop("O2", jacobi_consumer(ci, last=(it == K - 1)))
                    # yT from z halves (bf16 copies of y)
                    for t in range(NT):
                        transpose_to(yT_re, z_own[:, t, 0:F], t)
                        transpose_to(yT_im, z_own[:, t, F:F2], t)
                    dense_chunks([wbase + 1 + 2 * j, wbase + 2 + 2 * j],
                                 [yT_re, yT_im], first=False)

                if ci == 0:
                    # relu -> bf16, transpose back to node-major, reseed state
                    r_bf = pp.tile([128, NT * F], BF16, tag="rbf")
                    nc.vector.tensor_scalar_max(r_bf[:], out_acc[:], 0.0)
                    nc.vector.memset(y_im[:], 0.0)
                    for t in range(NT):
                        pt = tr_ps.tile([128, 128], BF16, tag="trps")
                        nc.tensor.transpose(pt[:], r_bf[:, t * 128:(t + 1) * 128],
                                            ident[:])
                        nc.vector.tensor_copy(z_own[:, t, 0:F], pt[:])
                        nc.vector.memset(z_own[:, t, F:F2], 0.0)
                        nc.vector.tensor_copy(y_re[:, t, :], pt[:])
                        nc.sync.dma_start(zin[t * 128:(t + 1) * 128, :],
                                          z_own[:, t, :])
                else:
                    # x2 = relu(out_acc), feature-major f32 -> DRAM
                    res = pp.tile([128, NT * F], F32, tag="res")
                    nc.vector.tensor_scalar_max(res[:], out_acc[:], 0.0)
                    nc.sync.dma_start(xfeat_out[:], res[:])

    nc.compile()
    return nc


# ----------------------------------------------------------------------------
# entry point
# ----------------------------------------------------------------------------

def kernel(x, edge_index, W_real1, Wc1, W_real2, Wc2, h, alpha,
           pool_w, lin_W, lin_b):
    x = np.asarray(x, np.float32)
    edge_index = np.asarray(edge_index)
    row, col = edge_index[0].astype(np.int64), edge_index[1].astype(np.int64)

    node_core, node_tile, node_slot = _assign_nodes(row, col)
    GT, tabs = _build_edge_tables(row, col, node_core, node_tile, node_slot)
    if "nc" not in _cache or _cache.get("GT") != GT:
        _cache["nc"] = _build(GT)
        _cache["GT"] = GT
    nc = _cache["nc"]

    deg = np.bincount(row, minlength=N).astype(np.float64)

    # per-node scale columns, both convs
    scal = np.zeros((NCORES, 128, 2 * NSCAL * NT), np.float32)
    for ci in range(2):
        sc = _scales(deg, float(np.asarray(h)[ci]), float(np.asarray(alpha)[ci]))
        for k, name in enumerate(SCAL_NAMES):
            cols = _shard_cols(sc[name].astype(np.float32),
                               node_core, node_tile, node_slot)
            scal[:, :, (ci * NSCAL + k) * NT:(ci * NSCAL + k + 1) * NT] = cols

    # weights: lhsT layouts [cin, cout] bf16; imag pre-negated
    def T16(w):
        return np.ascontiguousarray(w.T).astype(ml_dtypes.bfloat16)
    wts = np.zeros((128, 10 * 128), ml_dtypes.bfloat16)
    packs = [T16(W_real1), T16(Wc1[0, :, :, 0]), T16(-Wc1[0, :, :, 1]),
             T16(Wc1[1, :, :, 0]), T16(-Wc1[1, :, :, 1]),
             T16(W_real2), T16(Wc2[0, :, :, 0]), T16(-Wc2[0, :, :, 1]),
             T16(Wc2[1, :, :, 0]), T16(-Wc2[1, :, :, 1])]
    for k, w in enumerate(packs):
        wts[:, k * 128:(k + 1) * 128] = w

    xn = _node_major(x, node_core, node_tile, node_slot)  # [NCORES,128,NT,F]
    xz = np.zeros((NCORES, 128, NT, F2), ml_dtypes.bfloat16)
    xz[:, :, :, :F] = xn.astype(ml_dtypes.bfloat16)
    y0 = xn.reshape(NCORES, 128, NT * F)

    ident = np.eye(128, dtype=ml_dtypes.bfloat16)

    (idx1, S1), (idx2, S2) = tabs["O1"], tabs["O2"]
    in_maps = []
    for c in range(NCORES):
        in_maps.append({
            "xz": xz[c], "y0": y0[c],
            "idx1": idx1[c], "idx2": idx2[c],
            "s1": S1[c], "s2": S2[c],
            "scal": scal[c], "wts": wts, "ident": ident,
        })

    import os
    trace = os.environ.get("KERNEL_TRACE", "0") == "1"
    res = run_bass_kernel_spmd(nc, in_maps, core_ids=list(range(NCORES)),
                               trace=trace)
    _cache["last_results"] = res

    # unshard x2: xfeat[c][o, t*128+p] -> x2[node at (c,t,p), o]
    stk = np.stack([np.asarray(res.results[c]["xfeat"]).reshape(128, NT * F)
                    for c in range(NCORES)])
    x2 = stk[node_core, :, node_tile * 128 + node_slot].astype(np.float32)

    # host tail: tanh score, top-k (stable ties), weighted mean, linear
    pw = np.asarray(pool_w, np.float32)
    score = np.tanh((x2 @ pw) / np.linalg.norm(pw)).astype(np.float32)
    kpool = int(np.ceil(RATIO * N))
    idx = np.argsort(-score, kind="stable")[:kpool]
    x_sel = x2[idx] * score[idx][:, None]
    pooled = x_sel.mean(axis=0, keepdims=True).astype(np.float32)
    return (pooled @ np.asarray(lin_W, np.float32).T
            + np.asarray(lin_b, np.float32)).astype(np.float32)

